# revision 26
# baseline (speedup 1.0000x reference)
"""Multi-head attention (B=2, S=2048, E=1024, H=16, D=64) on 8 TRN2 cores.

Sharding: core c = b*4 + g  →  batch b ∈ {0,1}, head-group g ∈ {0..3}
(4 heads = 256 embed columns per group).  Each core computes its group's
Q/K/V projections, attention, and the partial output projection
(out^T [1024, 2048], the Wo[:, group]-contracted context).  Host sums the
4 group partials per batch, transposes, and adds bo.

Layout (matmul inputs bf16, all accumulation fp32 in PSUM):
- x is passed pre-transposed per batch: xT [1024, 2048] (e on partitions).
- Qᵀ/Kᵀ are head-dim-major [256, 2048]: head h of the group lives at
  partition rows (h%2)*64 of tile h//2.
- V is token-major, stored per 128-token tile as [128, 4*65]: head h at
  cols 65h..65h+63, col 65h+64 = 1.0 (ones column).
- scores are computed transposed (k on partitions, q free); softmax has
  no max-subtraction (scores ∈ ±2.8 for this input distribution).
- attn·V: lhsT = expᵀ chunk [k,128q], rhs = [V|1] [k,65] → PSUM [128q, 65]
  whose column 64 is the softmax denominator (per-partition scalar), so
  normalization is reciprocal + tensor_scalar_mul.
- context (q-major) is PE-transposed in 128x128 blocks for the output
  projection.
"""

import sys

import numpy as np

_REPO = "/opt/trn_rl_repo"
if _REPO not in sys.path:
    sys.path.insert(0, _REPO)

B, S, E = 2, 2048, 1024
HEADS, D = 16, 64
GROUPS = 4            # head groups (one per core within a batch)
HG = HEADS // GROUPS  # 4 heads per group
FG = HG * D           # 256 embed columns per group
SCALE = D ** -0.5     # 0.125

PF = 128              # partition tile
QC = 512              # free-dim chunk per matmul
NE = E // PF          # 8 contraction chunks over embed
NQ = S // QC          # 4 q chunks
NK = S // PF          # 16 k tiles
NS = S // PF          # 16 token tiles
NF = E // PF          # 8 output-feature tiles

_NC_CACHE = None


def _build_nc():
    """Build (once) the single-core Bass/Tile program run SPMD on all 8 cores."""
    global _NC_CACHE
    if _NC_CACHE is not None:
        return _NC_CACHE

    import concourse.bass as bass
    import concourse.tile as tile
    from concourse import bacc, mybir
    from concourse.masks import make_identity

    f32 = mybir.dt.float32
    bf16 = mybir.dt.bfloat16
    Exp = mybir.ActivationFunctionType.Exp
    ts = bass.ts

    nc = bacc.Bacc("TRN2", target_bir_lowering=False, debug=False)

    xT_d = nc.declare_dram_parameter("xT", [E, S], bf16, isOutput=False)
    wqT_d = nc.declare_dram_parameter("wqT", [E, FG], bf16, isOutput=False)
    wkT_d = nc.declare_dram_parameter("wkT", [E, FG], bf16, isOutput=False)
    wvT_d = nc.declare_dram_parameter("wvT", [E, FG], bf16, isOutput=False)
    woT_d = nc.declare_dram_parameter("woT", [FG, E], bf16, isOutput=False)
    bq_d = nc.declare_dram_parameter("bq2", [PF, 2], f32, isOutput=False)
    bk_d = nc.declare_dram_parameter("bk2", [PF, 2], f32, isOutput=False)
    bv_d = nc.declare_dram_parameter("bv1", [1, FG], f32, isOutput=False)
    outT_d = nc.declare_dram_parameter("outT", [E, S], bf16, isOutput=True)

    from contextlib import ExitStack

    _stack = ExitStack()
    stack_enter = _stack.enter_context
    with tile.TileContext(nc) as tc:
        with (
            tc.tile_pool(name="w", bufs=1) as pw,
            tc.tile_pool(name="qk", bufs=1) as pqk,
            tc.tile_pool(name="vpool", bufs=1) as pv,
            tc.tile_pool(name="ctx", bufs=1) as pctx,
        ):
            # ---- resident weights / biases -------------------------------
            wq_sb = [pw.tile([PF, FG], bf16, tag=f"wq{e}", name=f"wq{e}") for e in range(NE)]
            wk_sb = [pw.tile([PF, FG], bf16, tag=f"wk{e}", name=f"wk{e}") for e in range(NE)]
            wv_sb = [pw.tile([PF, FG], bf16, tag=f"wv{e}", name=f"wv{e}") for e in range(NE)]
            wo_sb = [pw.tile([PF, E], bf16, tag=f"wo{e}", name=f"wo{e}") for e in range(FG // PF)]
            bq_sb = pw.tile([PF, 2], f32, tag="bq")
            bk_sb = pw.tile([PF, 2], f32, tag="bk")
            bv_row = pw.tile([1, FG], f32, tag="bvr")
            bv_sb = pw.tile([PF, FG], f32, tag="bvf")
            ident = pw.tile([PF, PF], bf16, tag="ident")
            nc.sync.dma_start(bq_sb[:], bq_d[:])
            nc.sync.dma_start(bk_sb[:], bk_d[:])
            nc.sync.dma_start(bv_row[:], bv_d[:])

            # ---- persistent activations ----------------------------------
            qt_sb = [pqk.tile([PF, S], bf16, tag=f"qt{t}", name=f"qt{t}") for t in range(2)]
            # Kᵀ in two zero-padded parity copies so the scores matmul can use
            # K=128 stationary operands (K=64 disables fast weight load):
            # ktz[t] = [even-head rows | 0], kto[t] = [0 | odd-head rows]
            ktz_sb = [pqk.tile([PF, S], bf16, tag=f"ktz{t}", name=f"ktz{t}") for t in range(2)]
            kto_sb = [pqk.tile([PF, S], bf16, tag=f"kto{t}", name=f"kto{t}") for t in range(2)]
            # V token-tiles: [128 tokens, 4 heads x (64 + ones)]
            v_sb = [pv.tile([PF, HG * (D + 1)], bf16, tag=f"v{st}", name=f"v{st}") for st in range(NS)]
            # context, q-major per 128-token tile: [128, 4*64]
            ctx_sb = [pctx.tile([PF, FG], bf16, tag=f"ctx{qt}", name=f"ctx{qt}") for qt in range(NS)]
            # transposed context for the output projection
            ctxT_sb = [pctx.tile([PF, S], bf16, tag=f"ctxT{j}", name=f"ctxT{j}") for j in range(2)]


            # ---- phase 1: Q/K/V projections ------------------------------
            _stack.__enter__()
            pe = stack_enter(tc.tile_pool(name="et", bufs=36))
            pn = stack_enter(tc.tile_pool(name="nrm", bufs=4))
            po_sb = stack_enter(tc.tile_pool(name="owork", bufs=8))
            _p2 = ExitStack()
            _p2.__enter__()
            ppss = _p2.enter_context(tc.tile_pool(name="pss", bufs=2, space="PSUM"))
            _p1 = ExitStack()
            _p1.__enter__()
            px = _p1.enter_context(tc.tile_pool(name="xt", bufs=1))
            pps1 = _p1.enter_context(tc.tile_pool(name="ps1", bufs=2, space="PSUM"))
            pps1v = _p1.enter_context(tc.tile_pool(name="ps1v", bufs=2, space="PSUM"))
            if True:
                x_sb = [px.tile([PF, S], bf16, tag=f"x{e}", name=f"x{e}") for e in range(NE)]
                # interleave K-weight and first x-column DMAs: the first
                # accumulation chain's inputs arrive asap
                # spread the head's DMA issue across idle sequencers —
                # descriptor generation costs ~1.6us per dma_start per queue
                # spread the head's DMA issue across three idle sequencers —
                # descriptor generation costs ~1.6us per dma_start per queue,
                # so the issue rate (not bus bandwidth) paces the head
                for e in range(NE):
                    nc.sync.dma_start(x_sb[e][:, ts(0, QC)], xT_d[ts(e, PF), ts(0, QC)])
                    nc.scalar.dma_start(wk_sb[e][:], wkT_d[ts(e, PF), :])
                    nc.gpsimd.dma_start(wq_sb[e][:], wqT_d[ts(e, PF), :])
                for e in range(NE):
                    eng = (nc.sync, nc.gpsimd, nc.scalar)[e % 3]
                    eng.dma_start(x_sb[e][:, ts(1, QC)], xT_d[ts(e, PF), ts(1, QC)])
                # setup ops on the pool engine queue after its critical DMAs
                for t in range(2):
                    nc.gpsimd.memset(ktz_sb[t][D:PF, :], 0.0)
                    nc.gpsimd.memset(kto_sb[t][0:D, :], 0.0)
                nc.gpsimd.partition_broadcast(bv_sb[:], bv_row[:])
                make_identity(nc, ident[:])
                for e in range(NE):
                    eng = nc.gpsimd if e % 2 == 0 else nc.sync
                    eng.dma_start(wv_sb[e][:], wvT_d[ts(e, PF), :])
                    eng.dma_start(x_sb[e][:, 2 * QC:4 * QC], xT_d[ts(e, PF), 2 * QC:4 * QC])
                for e in range(FG // PF):
                    nc.sync.dma_start(wo_sb[e][:], woT_d[ts(e, PF), :])

                def project_qk(w_sb, b_sb, o_sb, t, c):
                    ps = pps1.tile([PF, QC], f32, tag="ps1", name="ps1")
                    for e in range(NE):
                        nc.tensor.matmul(
                            ps[:],
                            w_sb[e][:, ts(t, PF)],
                            x_sb[e][:, ts(c, QC)],
                            start=(e == 0),
                            stop=(e == NE - 1),
                        )
                    if o_sb is None:  # K: split into the parity copies
                        nc.vector.tensor_scalar_add(
                            ktz_sb[t][0:D, ts(c, QC)], ps[0:D, :], b_sb[0:D, t : t + 1]
                        )
                        nc.vector.tensor_scalar_add(
                            kto_sb[t][D:PF, ts(c, QC)], ps[D:PF, :], b_sb[D:PF, t : t + 1]
                        )
                    else:
                        nc.vector.tensor_scalar_add(
                            o_sb[t][:, ts(c, QC)], ps[:], b_sb[:, t : t + 1]
                        )

                def emit_scores_kt(half, h, et, kt, split_exp=False):
                    t = h // 2
                    k_sb = ktz_sb[t] if h % 2 == 0 else kto_sb[t]
                    psc = ppss.tile([PF, 2, QC], f32, tag="pss", name="pss")
                    e_t = pe.tile([PF, 2, QC], bf16, tag="et", name="et")
                    for j in range(2):
                        nc.tensor.matmul(
                            psc[:, j, :],
                            k_sb[:, ts(kt, PF)],
                            qt_sb[t][:, ts(half * 2 + j, QC)],
                            start=True,
                            stop=True,
                        )
                        if split_exp:
                            nc.scalar.activation(
                                e_t[:, j, :], psc[:, j, :], Exp, scale=SCALE
                            )
                    if not split_exp:
                        nc.scalar.activation(e_t[:], psc[:], Exp, scale=SCALE)
                    et.append(e_t)

                def emit_scores(half, h, et=None, kts=None):
                    if et is None:
                        et = []
                    for kt in kts if kts is not None else range(NK):
                        emit_scores_kt(half, h, et, kt)
                    return et

                def emit_attnv_qs(half, h, et, qs):
                    j, sub = qs // 4, qs % 4
                    po = ppso.tile([PF, D + 1], f32, tag="pso", name="pso")
                    for kt in range(NK):
                        nc.tensor.matmul(
                            po[:],
                            et[kt][:, j, ts(sub, PF)],
                            v_sb[kt][:, h * (D + 1) : (h + 1) * (D + 1)],
                            start=(kt == 0),
                            stop=(kt == NK - 1),
                        )
                    qt = half * 8 + qs
                    r = pn.tile([PF, 1], f32, tag="r", name="r")
                    nc.vector.reciprocal(r[:], po[:, D : D + 1])
                    nc.vector.tensor_scalar_mul(
                        ctx_sb[qt][:, ts(h, D)], po[:, 0:D], r[:]
                    )

                def emit_attnv(half, h, et):
                    for qs in range(8):
                        emit_attnv_qs(half, h, et, qs)

                def emit_transpose_qt(qt):
                    for j2 in range(2):
                        ptr = pptr.tile([PF, PF], bf16, tag="ptr", name="ptr")
                        nc.tensor.transpose(
                            ptr[:], ctx_sb[qt][:, ts(j2, PF)], ident[:]
                        )
                        nc.vector.tensor_copy(ctxT_sb[j2][:, ts(qt, PF)], ptr[:])

                def emit_transposes(half, qts=None):
                    for qt in qts if qts is not None else range(half * 8, half * 8 + 8):
                        emit_transpose_qt(qt)

                def emit_v(st):
                    ps = pps1v.tile([PF, FG], f32, tag="ps1v", name="ps1v")
                    for e in range(NE):
                        nc.tensor.matmul(
                            ps[:],
                            x_sb[e][:, ts(st, PF)],
                            wv_sb[e][:],
                            start=(e == 0),
                            stop=(e == NE - 1),
                        )
                    nc.vector.memset(v_sb[st][:], 1.0)
                    for h in range(HG):
                        nc.vector.tensor_add(
                            v_sb[st][:, h * (D + 1) : h * (D + 1) + D],
                            ps[:, ts(h, D)],
                            bv_sb[:, ts(h, D)],
                        )

                # interleave K-tile-0 projections with the first combo's
                # scores so the exp stream (ACT, the bottleneck) starts asap;
                # V-projection groups are spread through the chain so all of
                # V is done by the time the first attn-V group runs
                project_qk(wk_sb, bk_sb, None, 0, 0)
                project_qk(wq_sb, bq_sb, qt_sb, 0, 0)
                project_qk(wq_sb, bq_sb, qt_sb, 0, 1)
                et00 = []
                for kt in range(4):
                    emit_scores_kt(0, 0, et00, kt, split_exp=True)
                for c in range(1, NQ):
                    project_qk(wk_sb, bk_sb, None, 0, c)
                    for st in range(4 * (c - 1), 4 * c):
                        emit_v(st)
                    for kt in range(4 * c, 4 * c + 4):
                        emit_scores_kt(0, 0, et00, kt, split_exp=True)
                # interleave combo (0,1) scores (one kt per group) and
                # combo (0,0) attn-V through the phase-1 tail so neither ACT
                # nor PE drains while K-t1/Q-t1/V/Q-rest run
                et01 = []
                av00 = [0]
                tail = (
                    [lambda st=st: emit_v(st) for st in range(12, 16)]
                    + [lambda c=c: project_qk(wk_sb, bk_sb, None, 1, c) for c in range(NQ)]
                    + [lambda c=c: project_qk(wq_sb, bq_sb, qt_sb, 1, c) for c in range(2)]
                    + [
                        lambda t=t, c=c: project_qk(wq_sb, bq_sb, qt_sb, t, c)
                        for t in range(2)
                        for c in range(2, NQ)
                    ]
                )
                for fn in tail:
                    fn()
                    if len(et01) < NK:
                        emit_scores_kt(0, 1, et01, len(et01))
                while len(et01) < NK:
                    emit_scores_kt(0, 1, et01, len(et01))

                # ---- phase 2: attention (pipelined one combo ahead) ------
                _p1.close()
                ppso = _p2.enter_context(tc.tile_pool(name="pso", bufs=3, space="PSUM"))
                pptr = _p2.enter_context(tc.tile_pool(name="ptr", bufs=1, space="PSUM"))
                Copy = mybir.ActivationFunctionType.Copy
                oidx = [0]

                def emit_outproj_group(c, ft):
                    ps = ppso.tile([PF, QC], f32, tag="pso", name="pso")
                    for e in range(FG // PF):
                        nc.tensor.matmul(
                            ps[:],
                            wo_sb[e][:, ts(ft, PF)],
                            ctxT_sb[e][:, ts(c, QC)],
                            start=(e == 0),
                            stop=(e == FG // PF - 1),
                        )
                    ot = po_sb.tile([PF, QC], bf16, tag="ot", name="ot")
                    if oidx[0] % 2 == 0:
                        nc.vector.tensor_copy(ot[:], ps[:])
                    else:
                        nc.scalar.activation(ot[:], ps[:], Copy)
                    oidx[0] += 1
                    eng = nc.sync if oidx[0] % 2 == 0 else nc.gpsimd
                    eng.dma_start(outT_d[ts(ft, PF), ts(c, QC)], ot[:])

                def emit_outproj(c):
                    for ft in range(NF):
                        emit_outproj_group(c, ft)

                while av00[0] < 8:
                    emit_attnv_qs(0, 0, et00, av00[0])
                    av00[0] += 1

                sc_stream = [(0, 2), (0, 3), (1, 0), (1, 1), (1, 2), (1, 3)]
                av_stream = [(0, 1), (0, 2), (0, 3), (1, 0), (1, 1), (1, 2), (1, 3)]
                ets = {(0, 1): et01}
                pending = []
                extras = {
                    (0, 3): [
                        lambda: pending.extend(
                            (lambda qt=qt: emit_transpose_qt(qt)) for qt in range(8)
                        )
                    ],
                    (1, 0): [
                        lambda: pending.extend(
                            (lambda ft=ft: emit_outproj_group(0, ft)) for ft in range(NF)
                        )
                    ],
                    (1, 1): [
                        lambda: pending.extend(
                            (lambda ft=ft: emit_outproj_group(1, ft)) for ft in range(NF)
                        )
                    ],
                }
                for i, av in enumerate(av_stream):
                    cur = sc_stream[i] if i < len(sc_stream) else None
                    if cur is not None:
                        ets[cur] = []
                    for kt in range(NK):
                        if cur is not None:
                            emit_scores_kt(cur[0], cur[1], ets[cur], kt)
                        if kt % 2 == 0 and pending:
                            pending.pop(0)()
                        if kt % 2 == 1:
                            emit_attnv_qs(av[0], av[1], ets[av], kt // 2)
                            # tail: peel transposes/out-proj in as soon as
                            # their context tiles complete
                            if av == (1, 3):
                                if kt == 7:
                                    emit_transposes(1, qts=range(8, 12))
                                    pending.extend(
                                        (lambda ft=ft: emit_outproj_group(2, ft))
                                        for ft in range(NF)
                                    )
                                elif kt > 8:
                                    emit_transpose_qt(12 + kt // 2 - 4)
                    for fn in extras.get(av, ()):
                        fn()
                while pending:
                    pending.pop(0)()
                emit_outproj(3)
                _p2.close()

            _stack.close()

    nc.compile()
    _NC_CACHE = nc
    return nc


def _in_maps(x, Wq, bq, Wk, bk, Wv, bv, Wo, bo):
    """Per-core input dicts: core c = b*4 + g."""
    import ml_dtypes

    f = np.float32
    b16 = ml_dtypes.bfloat16
    maps = []
    for b in range(B):
        xT = np.ascontiguousarray(x[b].T).astype(b16)
        for g in range(GROUPS):
            gs = g * FG
            sl = slice(gs, gs + FG)
            maps.append(
                {
                    "xT": xT,
                    "wqT": np.ascontiguousarray(Wq[sl, :].T).astype(b16),
                    "wkT": np.ascontiguousarray(Wk[sl, :].T).astype(b16),
                    "wvT": np.ascontiguousarray(Wv[sl, :].T).astype(b16),
                    "woT": np.ascontiguousarray(Wo[:, sl].T).astype(b16),
                    "bq2": np.ascontiguousarray(bq[sl].reshape(2, PF).T, dtype=f),
                    "bk2": np.ascontiguousarray(bk[sl].reshape(2, PF).T, dtype=f),
                    "bv1": np.ascontiguousarray(bv[sl].reshape(1, FG), dtype=f),
                }
            )
    return maps


def _assemble(results, bo):
    out = np.empty((B, S, E), dtype=np.float32)
    for b in range(B):
        acc = results[b * GROUPS]["outT"].astype(np.float32, copy=True)
        for g in range(1, GROUPS):
            acc += results[b * GROUPS + g]["outT"]
        out[b] = acc.T + bo.astype(np.float32)
    return out


def kernel(x, Wq, bq, Wk, bk, Wv, bv, Wo, bo):
    from concourse.bass_utils import run_bass_kernel_spmd

    nc = _build_nc()
    maps = _in_maps(x, Wq, bq, Wk, bk, Wv, bv, Wo, bo)
    res = run_bass_kernel_spmd(nc, maps, core_ids=list(range(8)))
    return _assemble(res.results, np.asarray(bo))


# revision 27
# speedup vs baseline: 1.0049x; 1.0049x over previous
"""Multi-head attention (B=2, S=2048, E=1024, H=16, D=64) on 8 TRN2 cores.

Sharding: core c = b*4 + g  →  batch b ∈ {0,1}, head-group g ∈ {0..3}
(4 heads = 256 embed columns per group).  Each core computes its group's
Q/K/V projections, attention, and the partial output projection
(out^T [1024, 2048], the Wo[:, group]-contracted context).  Host sums the
4 group partials per batch, transposes, and adds bo.

Layout (matmul inputs bf16, all accumulation fp32 in PSUM):
- x is passed pre-transposed per batch: xT [1024, 2048] (e on partitions).
- Qᵀ/Kᵀ are head-dim-major [256, 2048]: head h of the group lives at
  partition rows (h%2)*64 of tile h//2.
- V is token-major, stored per 128-token tile as [128, 4*65]: head h at
  cols 65h..65h+63, col 65h+64 = 1.0 (ones column).
- scores are computed transposed (k on partitions, q free); softmax has
  no max-subtraction (scores ∈ ±2.8 for this input distribution).
- attn·V: lhsT = expᵀ chunk [k,128q], rhs = [V|1] [k,65] → PSUM [128q, 65]
  whose column 64 is the softmax denominator (per-partition scalar), so
  normalization is reciprocal + tensor_scalar_mul.
- context (q-major) is PE-transposed in 128x128 blocks for the output
  projection.
"""

import sys

import numpy as np

_REPO = "/opt/trn_rl_repo"
if _REPO not in sys.path:
    sys.path.insert(0, _REPO)

B, S, E = 2, 2048, 1024
HEADS, D = 16, 64
GROUPS = 4            # head groups (one per core within a batch)
HG = HEADS // GROUPS  # 4 heads per group
FG = HG * D           # 256 embed columns per group
SCALE = D ** -0.5     # 0.125

PF = 128              # partition tile
QC = 512              # free-dim chunk per matmul
NE = E // PF          # 8 contraction chunks over embed
NQ = S // QC          # 4 q chunks
NK = S // PF          # 16 k tiles
NS = S // PF          # 16 token tiles
NF = E // PF          # 8 output-feature tiles

_NC_CACHE = None


def _build_nc():
    """Build (once) the single-core Bass/Tile program run SPMD on all 8 cores."""
    global _NC_CACHE
    if _NC_CACHE is not None:
        return _NC_CACHE

    import concourse.bass as bass
    import concourse.tile as tile
    from concourse import bacc, mybir
    from concourse.masks import make_identity

    f32 = mybir.dt.float32
    bf16 = mybir.dt.bfloat16
    Exp = mybir.ActivationFunctionType.Exp
    ts = bass.ts

    nc = bacc.Bacc("TRN2", target_bir_lowering=False, debug=False)

    xT_d = nc.declare_dram_parameter("xT", [E, S], bf16, isOutput=False)
    wqT_d = nc.declare_dram_parameter("wqT", [E, FG], bf16, isOutput=False)
    wkT_d = nc.declare_dram_parameter("wkT", [E, FG], bf16, isOutput=False)
    wvT_d = nc.declare_dram_parameter("wvT", [E, FG], bf16, isOutput=False)
    woT_d = nc.declare_dram_parameter("woT", [FG, E], bf16, isOutput=False)
    bq_d = nc.declare_dram_parameter("bq2", [PF, 2], f32, isOutput=False)
    bk_d = nc.declare_dram_parameter("bk2", [PF, 2], f32, isOutput=False)
    bv_d = nc.declare_dram_parameter("bv1", [1, FG], f32, isOutput=False)
    outT_d = nc.declare_dram_parameter("outT", [E, S], bf16, isOutput=True)

    from contextlib import ExitStack

    _stack = ExitStack()
    stack_enter = _stack.enter_context
    with tile.TileContext(nc) as tc:
        with (
            tc.tile_pool(name="w", bufs=1) as pw,
            tc.tile_pool(name="qk", bufs=1) as pqk,
            tc.tile_pool(name="vpool", bufs=1) as pv,
            tc.tile_pool(name="ctx", bufs=1) as pctx,
        ):
            # ---- resident weights / biases -------------------------------
            wq_sb = [pw.tile([PF, FG], bf16, tag=f"wq{e}", name=f"wq{e}") for e in range(NE)]
            wk_sb = [pw.tile([PF, FG], bf16, tag=f"wk{e}", name=f"wk{e}") for e in range(NE)]
            wv_sb = [pw.tile([PF, FG], bf16, tag=f"wv{e}", name=f"wv{e}") for e in range(NE)]
            wo_sb = [pw.tile([PF, E], bf16, tag=f"wo{e}", name=f"wo{e}") for e in range(FG // PF)]
            bq_sb = pw.tile([PF, 2], f32, tag="bq")
            bk_sb = pw.tile([PF, 2], f32, tag="bk")
            bv_row = pw.tile([1, FG], f32, tag="bvr")
            bv_sb = pw.tile([PF, FG], f32, tag="bvf")
            ident = pw.tile([PF, PF], bf16, tag="ident")
            nc.sync.dma_start(bq_sb[:], bq_d[:])
            nc.sync.dma_start(bk_sb[:], bk_d[:])
            nc.sync.dma_start(bv_row[:], bv_d[:])

            # ---- persistent activations ----------------------------------
            qt_sb = [pqk.tile([PF, S], bf16, tag=f"qt{t}", name=f"qt{t}") for t in range(2)]
            # Kᵀ in two zero-padded parity copies so the scores matmul can use
            # K=128 stationary operands (K=64 disables fast weight load):
            # ktz[t] = [even-head rows | 0], kto[t] = [0 | odd-head rows]
            ktz_sb = [pqk.tile([PF, S], bf16, tag=f"ktz{t}", name=f"ktz{t}") for t in range(2)]
            kto_sb = [pqk.tile([PF, S], bf16, tag=f"kto{t}", name=f"kto{t}") for t in range(2)]
            # V token-tiles: [128 tokens, 4 heads x (64 + ones)]
            v_sb = [pv.tile([PF, HG * (D + 1)], bf16, tag=f"v{st}", name=f"v{st}") for st in range(NS)]
            # context, q-major per 128-token tile: [128, 4*64]
            ctx_sb = [pctx.tile([PF, FG], bf16, tag=f"ctx{qt}", name=f"ctx{qt}") for qt in range(NS)]
            # transposed context for the output projection
            ctxT_sb = [pctx.tile([PF, S], bf16, tag=f"ctxT{j}", name=f"ctxT{j}") for j in range(2)]


            # ---- phase 1: Q/K/V projections ------------------------------
            _stack.__enter__()
            pe = stack_enter(tc.tile_pool(name="et", bufs=36))
            pn = stack_enter(tc.tile_pool(name="nrm", bufs=4))
            po_sb = stack_enter(tc.tile_pool(name="owork", bufs=8))
            _p2 = ExitStack()
            _p2.__enter__()
            ppss = _p2.enter_context(tc.tile_pool(name="pss", bufs=2, space="PSUM"))
            _p1 = ExitStack()
            _p1.__enter__()
            px = _p1.enter_context(tc.tile_pool(name="xt", bufs=1))
            pps1 = _p1.enter_context(tc.tile_pool(name="ps1", bufs=2, space="PSUM"))
            pps1v = _p1.enter_context(tc.tile_pool(name="ps1v", bufs=2, space="PSUM"))
            if True:
                x_sb = [px.tile([PF, S], bf16, tag=f"x{e}", name=f"x{e}") for e in range(NE)]
                # interleave K-weight and first x-column DMAs: the first
                # accumulation chain's inputs arrive asap
                # spread the head's DMA issue across idle sequencers —
                # descriptor generation costs ~1.6us per dma_start per queue
                # spread the head's DMA issue across three idle sequencers —
                # descriptor generation costs ~1.6us per dma_start per queue,
                # so the issue rate (not bus bandwidth) paces the head
                for e in range(NE):
                    nc.sync.dma_start(x_sb[e][:, ts(0, QC)], xT_d[ts(e, PF), ts(0, QC)])
                    nc.scalar.dma_start(wk_sb[e][:], wkT_d[ts(e, PF), :])
                    nc.gpsimd.dma_start(wq_sb[e][:], wqT_d[ts(e, PF), :])
                for e in range(NE):
                    eng = nc.sync if e % 2 == 0 else nc.gpsimd
                    eng.dma_start(x_sb[e][:, ts(1, QC)], xT_d[ts(e, PF), ts(1, QC)])
                # setup ops on the pool engine queue after its critical DMAs
                for t in range(2):
                    nc.gpsimd.memset(ktz_sb[t][D:PF, :], 0.0)
                    nc.gpsimd.memset(kto_sb[t][0:D, :], 0.0)
                nc.gpsimd.partition_broadcast(bv_sb[:], bv_row[:])
                make_identity(nc, ident[:])
                for e in range(NE):
                    eng = nc.gpsimd if e % 2 == 0 else nc.sync
                    eng.dma_start(wv_sb[e][:], wvT_d[ts(e, PF), :])
                    eng.dma_start(x_sb[e][:, 2 * QC:4 * QC], xT_d[ts(e, PF), 2 * QC:4 * QC])
                for e in range(FG // PF):
                    nc.sync.dma_start(wo_sb[e][:], woT_d[ts(e, PF), :])

                def project_qk(w_sb, b_sb, o_sb, t, c):
                    ps = pps1.tile([PF, QC], f32, tag="ps1", name="ps1")
                    for e in range(NE):
                        nc.tensor.matmul(
                            ps[:],
                            w_sb[e][:, ts(t, PF)],
                            x_sb[e][:, ts(c, QC)],
                            start=(e == 0),
                            stop=(e == NE - 1),
                        )
                    if o_sb is None:  # K: split into the parity copies
                        nc.vector.tensor_scalar_add(
                            ktz_sb[t][0:D, ts(c, QC)], ps[0:D, :], b_sb[0:D, t : t + 1]
                        )
                        nc.vector.tensor_scalar_add(
                            kto_sb[t][D:PF, ts(c, QC)], ps[D:PF, :], b_sb[D:PF, t : t + 1]
                        )
                    else:
                        nc.vector.tensor_scalar_add(
                            o_sb[t][:, ts(c, QC)], ps[:], b_sb[:, t : t + 1]
                        )

                def emit_scores_kt(half, h, et, kt, split_exp=False):
                    t = h // 2
                    k_sb = ktz_sb[t] if h % 2 == 0 else kto_sb[t]
                    psc = ppss.tile([PF, 2, QC], f32, tag="pss", name="pss")
                    e_t = pe.tile([PF, 2, QC], bf16, tag="et", name="et")
                    for j in range(2):
                        nc.tensor.matmul(
                            psc[:, j, :],
                            k_sb[:, ts(kt, PF)],
                            qt_sb[t][:, ts(half * 2 + j, QC)],
                            start=True,
                            stop=True,
                        )
                        if split_exp:
                            nc.scalar.activation(
                                e_t[:, j, :], psc[:, j, :], Exp, scale=SCALE
                            )
                    if not split_exp:
                        nc.scalar.activation(e_t[:], psc[:], Exp, scale=SCALE)
                    et.append(e_t)

                def emit_scores(half, h, et=None, kts=None):
                    if et is None:
                        et = []
                    for kt in kts if kts is not None else range(NK):
                        emit_scores_kt(half, h, et, kt)
                    return et

                def emit_attnv_qs(half, h, et, qs):
                    j, sub = qs // 4, qs % 4
                    po = ppso.tile([PF, D + 1], f32, tag="pso", name="pso")
                    for kt in range(NK):
                        nc.tensor.matmul(
                            po[:],
                            et[kt][:, j, ts(sub, PF)],
                            v_sb[kt][:, h * (D + 1) : (h + 1) * (D + 1)],
                            start=(kt == 0),
                            stop=(kt == NK - 1),
                        )
                    qt = half * 8 + qs
                    r = pn.tile([PF, 1], f32, tag="r", name="r")
                    nc.vector.reciprocal(r[:], po[:, D : D + 1])
                    nc.vector.tensor_scalar_mul(
                        ctx_sb[qt][:, ts(h, D)], po[:, 0:D], r[:]
                    )

                def emit_attnv(half, h, et):
                    for qs in range(8):
                        emit_attnv_qs(half, h, et, qs)

                def emit_transpose_qt(qt):
                    for j2 in range(2):
                        ptr = pptr.tile([PF, PF], bf16, tag="ptr", name="ptr")
                        nc.tensor.transpose(
                            ptr[:], ctx_sb[qt][:, ts(j2, PF)], ident[:]
                        )
                        nc.vector.tensor_copy(ctxT_sb[j2][:, ts(qt, PF)], ptr[:])

                def emit_transposes(half, qts=None):
                    for qt in qts if qts is not None else range(half * 8, half * 8 + 8):
                        emit_transpose_qt(qt)

                def emit_v(st):
                    ps = pps1v.tile([PF, FG], f32, tag="ps1v", name="ps1v")
                    for e in range(NE):
                        nc.tensor.matmul(
                            ps[:],
                            x_sb[e][:, ts(st, PF)],
                            wv_sb[e][:],
                            start=(e == 0),
                            stop=(e == NE - 1),
                        )
                    nc.vector.memset(v_sb[st][:], 1.0)
                    for h in range(HG):
                        nc.vector.tensor_add(
                            v_sb[st][:, h * (D + 1) : h * (D + 1) + D],
                            ps[:, ts(h, D)],
                            bv_sb[:, ts(h, D)],
                        )

                # interleave K-tile-0 projections with the first combo's
                # scores so the exp stream (ACT, the bottleneck) starts asap;
                # V-projection groups are spread through the chain so all of
                # V is done by the time the first attn-V group runs
                project_qk(wk_sb, bk_sb, None, 0, 0)
                project_qk(wq_sb, bq_sb, qt_sb, 0, 0)
                project_qk(wq_sb, bq_sb, qt_sb, 0, 1)
                et00 = []
                for kt in range(4):
                    emit_scores_kt(0, 0, et00, kt, split_exp=True)
                for c in range(1, NQ):
                    project_qk(wk_sb, bk_sb, None, 0, c)
                    for st in range(4 * (c - 1), 4 * c):
                        emit_v(st)
                    for kt in range(4 * c, 4 * c + 4):
                        emit_scores_kt(0, 0, et00, kt, split_exp=True)
                # interleave combo (0,1) scores (one kt per group) and
                # combo (0,0) attn-V through the phase-1 tail so neither ACT
                # nor PE drains while K-t1/Q-t1/V/Q-rest run
                et01 = []
                av00 = [0]
                tail = (
                    [lambda st=st: emit_v(st) for st in range(12, 16)]
                    + [lambda c=c: project_qk(wk_sb, bk_sb, None, 1, c) for c in range(NQ)]
                    + [lambda c=c: project_qk(wq_sb, bq_sb, qt_sb, 1, c) for c in range(2)]
                    + [
                        lambda t=t, c=c: project_qk(wq_sb, bq_sb, qt_sb, t, c)
                        for t in range(2)
                        for c in range(2, NQ)
                    ]
                )
                for fn in tail:
                    fn()
                    if len(et01) < NK:
                        emit_scores_kt(0, 1, et01, len(et01))
                while len(et01) < NK:
                    emit_scores_kt(0, 1, et01, len(et01))

                # ---- phase 2: attention (pipelined one combo ahead) ------
                _p1.close()
                ppso = _p2.enter_context(tc.tile_pool(name="pso", bufs=3, space="PSUM"))
                pptr = _p2.enter_context(tc.tile_pool(name="ptr", bufs=1, space="PSUM"))
                Copy = mybir.ActivationFunctionType.Copy
                oidx = [0]

                def emit_outproj_group(c, ft):
                    ps = ppso.tile([PF, QC], f32, tag="pso", name="pso")
                    for e in range(FG // PF):
                        nc.tensor.matmul(
                            ps[:],
                            wo_sb[e][:, ts(ft, PF)],
                            ctxT_sb[e][:, ts(c, QC)],
                            start=(e == 0),
                            stop=(e == FG // PF - 1),
                        )
                    ot = po_sb.tile([PF, QC], bf16, tag="ot", name="ot")
                    nc.vector.tensor_copy(ot[:], ps[:])
                    oidx[0] += 1
                    eng = nc.sync if oidx[0] % 2 == 0 else nc.gpsimd
                    eng.dma_start(outT_d[ts(ft, PF), ts(c, QC)], ot[:])

                def emit_outproj(c):
                    for ft in range(NF):
                        emit_outproj_group(c, ft)

                while av00[0] < 8:
                    emit_attnv_qs(0, 0, et00, av00[0])
                    av00[0] += 1

                sc_stream = [(0, 2), (0, 3), (1, 0), (1, 1), (1, 2), (1, 3)]
                av_stream = [(0, 1), (0, 2), (0, 3), (1, 0), (1, 1), (1, 2), (1, 3)]
                ets = {(0, 1): et01}
                pending = []
                extras = {
                    (0, 3): [
                        lambda: pending.extend(
                            (lambda qt=qt: emit_transpose_qt(qt)) for qt in range(8)
                        )
                    ],
                    (1, 0): [
                        lambda: pending.extend(
                            (lambda ft=ft: emit_outproj_group(0, ft)) for ft in range(NF)
                        )
                    ],
                    (1, 1): [
                        lambda: pending.extend(
                            (lambda ft=ft: emit_outproj_group(1, ft)) for ft in range(NF)
                        )
                    ],
                }
                for i, av in enumerate(av_stream):
                    cur = sc_stream[i] if i < len(sc_stream) else None
                    if cur is not None:
                        ets[cur] = []
                    for kt in range(NK):
                        if cur is not None:
                            emit_scores_kt(cur[0], cur[1], ets[cur], kt)
                        if kt % 2 == 0 and pending:
                            pending.pop(0)()
                        if kt % 2 == 1:
                            emit_attnv_qs(av[0], av[1], ets[av], kt // 2)
                            # tail: peel transposes/out-proj in as soon as
                            # their context tiles complete
                            if av == (1, 3):
                                if kt == 7:
                                    emit_transposes(1, qts=range(8, 12))
                                    pending.extend(
                                        (lambda ft=ft: emit_outproj_group(2, ft))
                                        for ft in range(NF)
                                    )
                                elif kt > 8:
                                    emit_transpose_qt(12 + kt // 2 - 4)
                    for fn in extras.get(av, ()):
                        fn()
                while pending:
                    pending.pop(0)()
                emit_outproj(3)
                _p2.close()

            _stack.close()

    nc.compile()
    _NC_CACHE = nc
    return nc


def _in_maps(x, Wq, bq, Wk, bk, Wv, bv, Wo, bo):
    """Per-core input dicts: core c = b*4 + g."""
    import ml_dtypes

    f = np.float32
    b16 = ml_dtypes.bfloat16
    maps = []
    for b in range(B):
        xT = np.ascontiguousarray(x[b].T).astype(b16)
        for g in range(GROUPS):
            gs = g * FG
            sl = slice(gs, gs + FG)
            maps.append(
                {
                    "xT": xT,
                    "wqT": np.ascontiguousarray(Wq[sl, :].T).astype(b16),
                    "wkT": np.ascontiguousarray(Wk[sl, :].T).astype(b16),
                    "wvT": np.ascontiguousarray(Wv[sl, :].T).astype(b16),
                    "woT": np.ascontiguousarray(Wo[:, sl].T).astype(b16),
                    "bq2": np.ascontiguousarray(bq[sl].reshape(2, PF).T, dtype=f),
                    "bk2": np.ascontiguousarray(bk[sl].reshape(2, PF).T, dtype=f),
                    "bv1": np.ascontiguousarray(bv[sl].reshape(1, FG), dtype=f),
                }
            )
    return maps


def _assemble(results, bo):
    out = np.empty((B, S, E), dtype=np.float32)
    for b in range(B):
        acc = results[b * GROUPS]["outT"].astype(np.float32, copy=True)
        for g in range(1, GROUPS):
            acc += results[b * GROUPS + g]["outT"]
        out[b] = acc.T + bo.astype(np.float32)
    return out


def kernel(x, Wq, bq, Wk, bk, Wv, bv, Wo, bo):
    from concourse.bass_utils import run_bass_kernel_spmd

    nc = _build_nc()
    maps = _in_maps(x, Wq, bq, Wk, bk, Wv, bv, Wo, bo)
    res = run_bass_kernel_spmd(nc, maps, core_ids=list(range(8)))
    return _assemble(res.results, np.asarray(bo))


# revision 28
# speedup vs baseline: 1.0223x; 1.0174x over previous
"""Multi-head attention (B=2, S=2048, E=1024, H=16, D=64) on 8 TRN2 cores.

Sharding: core c = b*4 + g  →  batch b ∈ {0,1}, head-group g ∈ {0..3}
(4 heads = 256 embed columns per group).  Each core computes its group's
Q/K/V projections, attention, and the partial output projection
(out^T [1024, 2048], the Wo[:, group]-contracted context).  Host sums the
4 group partials per batch, transposes, and adds bo.

Layout (matmul inputs bf16, all accumulation fp32 in PSUM):
- x is passed pre-transposed per batch: xT [1024, 2048] (e on partitions).
- Qᵀ/Kᵀ are head-dim-major [256, 2048]: head h of the group lives at
  partition rows (h%2)*64 of tile h//2.
- V is token-major, stored per 128-token tile as [128, 4*65]: head h at
  cols 65h..65h+63, col 65h+64 = 1.0 (ones column).
- scores are computed transposed (k on partitions, q free); softmax has
  no max-subtraction (scores ∈ ±2.8 for this input distribution).
- attn·V: lhsT = expᵀ chunk [k,128q], rhs = [V|1] [k,65] → PSUM [128q, 65]
  whose column 64 is the softmax denominator (per-partition scalar), so
  normalization is reciprocal + tensor_scalar_mul.
- context (q-major) is PE-transposed in 128x128 blocks for the output
  projection.
"""

import sys

import numpy as np

_REPO = "/opt/trn_rl_repo"
if _REPO not in sys.path:
    sys.path.insert(0, _REPO)

B, S, E = 2, 2048, 1024
HEADS, D = 16, 64
GROUPS = 4            # head groups (one per core within a batch)
HG = HEADS // GROUPS  # 4 heads per group
FG = HG * D           # 256 embed columns per group
SCALE = D ** -0.5     # 0.125

PF = 128              # partition tile
QC = 512              # free-dim chunk per matmul
NE = E // PF          # 8 contraction chunks over embed
NQ = S // QC          # 4 q chunks
NK = S // PF          # 16 k tiles
NS = S // PF          # 16 token tiles
NF = E // PF          # 8 output-feature tiles

_NC_CACHE = None


def _build_nc():
    """Build (once) the single-core Bass/Tile program run SPMD on all 8 cores."""
    global _NC_CACHE
    if _NC_CACHE is not None:
        return _NC_CACHE

    import concourse.bass as bass
    import concourse.tile as tile
    from concourse import bacc, mybir
    from concourse.masks import make_identity

    f32 = mybir.dt.float32
    bf16 = mybir.dt.bfloat16
    Exp = mybir.ActivationFunctionType.Exp
    ts = bass.ts

    nc = bacc.Bacc("TRN2", target_bir_lowering=False, debug=False)

    xT_d = nc.declare_dram_parameter("xT", [E, S], bf16, isOutput=False)
    wqT_d = nc.declare_dram_parameter("wqT", [E, FG], bf16, isOutput=False)
    wkT_d = nc.declare_dram_parameter("wkT", [E, FG], bf16, isOutput=False)
    wvT_d = nc.declare_dram_parameter("wvT", [E, FG], bf16, isOutput=False)
    woT_d = nc.declare_dram_parameter("woT", [FG, E], bf16, isOutput=False)
    bq_d = nc.declare_dram_parameter("bq2", [PF, 2], f32, isOutput=False)
    bk_d = nc.declare_dram_parameter("bk2", [PF, 2], f32, isOutput=False)
    bv_d = nc.declare_dram_parameter("bv1", [1, FG], f32, isOutput=False)
    outT_d = nc.declare_dram_parameter("outT", [E, S], bf16, isOutput=True)

    from contextlib import ExitStack

    _stack = ExitStack()
    stack_enter = _stack.enter_context
    with tile.TileContext(nc) as tc:
        with (
            tc.tile_pool(name="w", bufs=1) as pw,
            tc.tile_pool(name="qk", bufs=1) as pqk,
            tc.tile_pool(name="vpool", bufs=1) as pv,
            tc.tile_pool(name="ctx", bufs=1) as pctx,
        ):
            # ---- resident weights / biases -------------------------------
            wq_sb = [pw.tile([PF, FG], bf16, tag=f"wq{e}", name=f"wq{e}") for e in range(NE)]
            wk_sb = [pw.tile([PF, FG], bf16, tag=f"wk{e}", name=f"wk{e}") for e in range(NE)]
            wv_sb = [pw.tile([PF, FG], bf16, tag=f"wv{e}", name=f"wv{e}") for e in range(NE)]
            wo_sb = [pw.tile([PF, E], bf16, tag=f"wo{e}", name=f"wo{e}") for e in range(FG // PF)]
            bq_sb = pw.tile([PF, 2], f32, tag="bq")
            bk_sb = pw.tile([PF, 2], f32, tag="bk")
            bv_row = pw.tile([1, FG], f32, tag="bvr")
            bv_sb = pw.tile([PF, FG], f32, tag="bvf")
            ident = pw.tile([PF, PF], bf16, tag="ident")
            nc.sync.dma_start(bq_sb[:], bq_d[:])
            nc.sync.dma_start(bk_sb[:], bk_d[:])
            nc.sync.dma_start(bv_row[:], bv_d[:])

            # ---- persistent activations ----------------------------------
            qt_sb = [pqk.tile([PF, S], bf16, tag=f"qt{t}", name=f"qt{t}") for t in range(2)]
            # Kᵀ in two zero-padded parity copies so the scores matmul can use
            # K=128 stationary operands (K=64 disables fast weight load):
            # ktz[t] = [even-head rows | 0], kto[t] = [0 | odd-head rows]
            ktz_sb = [pqk.tile([PF, S], bf16, tag=f"ktz{t}", name=f"ktz{t}") for t in range(2)]
            kto_sb = [pqk.tile([PF, S], bf16, tag=f"kto{t}", name=f"kto{t}") for t in range(2)]
            # V token-tiles: [128 tokens, 4 heads x (64 + ones)]
            v_sb = [pv.tile([PF, HG * (D + 1)], bf16, tag=f"v{st}", name=f"v{st}") for st in range(NS)]
            # context, q-major per 128-token tile: [128, 4*64]
            ctx_sb = [pctx.tile([PF, FG], bf16, tag=f"ctx{qt}", name=f"ctx{qt}") for qt in range(NS)]
            # transposed context for the output projection
            ctxT_sb = [pctx.tile([PF, S], bf16, tag=f"ctxT{j}", name=f"ctxT{j}") for j in range(2)]


            # ---- phase 1: Q/K/V projections ------------------------------
            _stack.__enter__()
            pe = stack_enter(tc.tile_pool(name="et", bufs=36))
            pn = stack_enter(tc.tile_pool(name="nrm", bufs=4))
            po_sb = stack_enter(tc.tile_pool(name="owork", bufs=8))
            _p2 = ExitStack()
            _p2.__enter__()
            ppss = _p2.enter_context(tc.tile_pool(name="pss", bufs=2, space="PSUM"))
            _p1 = ExitStack()
            _p1.__enter__()
            px = _p1.enter_context(tc.tile_pool(name="xt", bufs=1))
            pps1 = _p1.enter_context(tc.tile_pool(name="ps1", bufs=2, space="PSUM"))
            pps1v = _p1.enter_context(tc.tile_pool(name="ps1v", bufs=2, space="PSUM"))
            if True:
                x_sb = [px.tile([PF, S], bf16, tag=f"x{e}", name=f"x{e}") for e in range(NE)]
                # interleave K-weight and first x-column DMAs: the first
                # accumulation chain's inputs arrive asap
                # spread the head's DMA issue across idle sequencers —
                # descriptor generation costs ~1.6us per dma_start per queue
                # spread the head's DMA issue across three idle sequencers —
                # descriptor generation costs ~1.6us per dma_start per queue,
                # so the issue rate (not bus bandwidth) paces the head
                for e in range(NE):
                    nc.sync.dma_start(x_sb[e][:, ts(0, QC)], xT_d[ts(e, PF), ts(0, QC)])
                    nc.scalar.dma_start(wk_sb[e][:], wkT_d[ts(e, PF), :])
                    nc.gpsimd.dma_start(wq_sb[e][:], wqT_d[ts(e, PF), :])
                for e in range(NE):
                    eng = nc.sync if e % 2 == 0 else nc.gpsimd
                    eng.dma_start(x_sb[e][:, ts(1, QC)], xT_d[ts(e, PF), ts(1, QC)])
                # setup ops on the pool engine queue after its critical DMAs
                for t in range(2):
                    nc.gpsimd.memset(ktz_sb[t][D:PF, :], 0.0)
                    nc.gpsimd.memset(kto_sb[t][0:D, :], 0.0)
                nc.gpsimd.partition_broadcast(bv_sb[:], bv_row[:])
                make_identity(nc, ident[:])
                for e in range(NE):
                    eng = nc.gpsimd if e % 2 == 0 else nc.sync
                    eng.dma_start(wv_sb[e][:], wvT_d[ts(e, PF), :])
                    eng.dma_start(x_sb[e][:, 2 * QC:4 * QC], xT_d[ts(e, PF), 2 * QC:4 * QC])
                for e in range(FG // PF):
                    nc.sync.dma_start(wo_sb[e][:], woT_d[ts(e, PF), :])

                def project_qk(w_sb, b_sb, o_sb, t, c):
                    ps = pps1.tile([PF, QC], f32, tag="ps1", name="ps1")
                    for e in range(NE):
                        nc.tensor.matmul(
                            ps[:],
                            w_sb[e][:, ts(t, PF)],
                            x_sb[e][:, ts(c, QC)],
                            start=(e == 0),
                            stop=(e == NE - 1),
                        )
                    if o_sb is None:  # K: split into the parity copies
                        nc.vector.tensor_scalar_add(
                            ktz_sb[t][0:D, ts(c, QC)], ps[0:D, :], b_sb[0:D, t : t + 1]
                        )
                        nc.vector.tensor_scalar_add(
                            kto_sb[t][D:PF, ts(c, QC)], ps[D:PF, :], b_sb[D:PF, t : t + 1]
                        )
                    else:
                        nc.vector.tensor_scalar_add(
                            o_sb[t][:, ts(c, QC)], ps[:], b_sb[:, t : t + 1]
                        )

                def emit_scores_kt(half, h, et, kt, split_exp=False):
                    t = h // 2
                    k_sb = ktz_sb[t] if h % 2 == 0 else kto_sb[t]
                    psc = ppss.tile([PF, 2, QC], f32, tag="pss", name="pss")
                    e_t = pe.tile([PF, 2, QC], bf16, tag="et", name="et")
                    for j in range(2):
                        nc.tensor.matmul(
                            psc[:, j, :],
                            k_sb[:, ts(kt, PF)],
                            qt_sb[t][:, ts(half * 2 + j, QC)],
                            start=True,
                            stop=True,
                        )
                        if split_exp:
                            nc.scalar.activation(
                                e_t[:, j, :], psc[:, j, :], Exp, scale=SCALE
                            )
                    if not split_exp:
                        nc.scalar.activation(e_t[:], psc[:], Exp, scale=SCALE)
                    et.append(e_t)

                def emit_scores(half, h, et=None, kts=None):
                    if et is None:
                        et = []
                    for kt in kts if kts is not None else range(NK):
                        emit_scores_kt(half, h, et, kt)
                    return et

                def emit_attnv_qs(half, h, et, qs):
                    j, sub = qs // 4, qs % 4
                    po = ppso.tile([PF, D + 1], f32, tag="pso", name="pso")
                    for kt in range(NK):
                        nc.tensor.matmul(
                            po[:],
                            et[kt][:, j, ts(sub, PF)],
                            v_sb[kt][:, h * (D + 1) : (h + 1) * (D + 1)],
                            start=(kt == 0),
                            stop=(kt == NK - 1),
                        )
                    qt = half * 8 + qs
                    r = pn.tile([PF, 1], f32, tag="r", name="r")
                    nc.vector.reciprocal(r[:], po[:, D : D + 1])
                    nc.vector.tensor_scalar_mul(
                        ctx_sb[qt][:, ts(h, D)], po[:, 0:D], r[:]
                    )

                def emit_attnv(half, h, et):
                    for qs in range(8):
                        emit_attnv_qs(half, h, et, qs)

                def emit_transpose_qt(qt):
                    for j2 in range(2):
                        ptr = pptr.tile([PF, PF], bf16, tag="ptr", name="ptr")
                        nc.tensor.transpose(
                            ptr[:], ctx_sb[qt][:, ts(j2, PF)], ident[:]
                        )
                        nc.vector.tensor_copy(ctxT_sb[j2][:, ts(qt, PF)], ptr[:])

                def emit_transposes(half, qts=None):
                    for qt in qts if qts is not None else range(half * 8, half * 8 + 8):
                        emit_transpose_qt(qt)

                def emit_v(st):
                    ps = pps1v.tile([PF, FG], f32, tag="ps1v", name="ps1v")
                    for e in range(NE):
                        nc.tensor.matmul(
                            ps[:],
                            x_sb[e][:, ts(st, PF)],
                            wv_sb[e][:],
                            start=(e == 0),
                            stop=(e == NE - 1),
                        )
                    nc.vector.memset(v_sb[st][:], 1.0)
                    for h in range(HG):
                        nc.vector.tensor_add(
                            v_sb[st][:, h * (D + 1) : h * (D + 1) + D],
                            ps[:, ts(h, D)],
                            bv_sb[:, ts(h, D)],
                        )

                # interleave K-tile-0 projections with the first combo's
                # scores so the exp stream (ACT, the bottleneck) starts asap;
                # V-projection groups are spread through the chain so all of
                # V is done by the time the first attn-V group runs
                project_qk(wk_sb, bk_sb, None, 0, 0)
                project_qk(wq_sb, bq_sb, qt_sb, 0, 0)
                project_qk(wq_sb, bq_sb, qt_sb, 0, 1)
                et00 = []
                for kt in range(4):
                    emit_scores_kt(0, 0, et00, kt, split_exp=True)
                for c in range(1, NQ):
                    project_qk(wk_sb, bk_sb, None, 0, c)
                    for st in range(4 * (c - 1), 4 * c):
                        emit_v(st)
                    for kt in range(4 * c, 4 * c + 4):
                        emit_scores_kt(0, 0, et00, kt, split_exp=True)
                # interleave combo (0,1) scores (one kt per group) and
                # combo (0,0) attn-V through the phase-1 tail so neither ACT
                # nor PE drains while K-t1/Q-t1/V/Q-rest run
                et01 = []
                av00 = [0]
                tail = (
                    [lambda st=st: emit_v(st) for st in range(12, 16)]
                    + [lambda c=c: project_qk(wk_sb, bk_sb, None, 1, c) for c in range(NQ)]
                    + [lambda c=c: project_qk(wq_sb, bq_sb, qt_sb, 1, c) for c in range(2)]
                )
                for fn in tail:
                    fn()
                    if len(et01) < NK:
                        emit_scores_kt(0, 1, et01, len(et01))
                while len(et01) < NK:
                    emit_scores_kt(0, 1, et01, len(et01))

                # ---- phase 2: attention (pipelined one combo ahead) ------
                _p1.close()
                ppso = _p2.enter_context(tc.tile_pool(name="pso", bufs=3, space="PSUM"))
                pptr = _p2.enter_context(tc.tile_pool(name="ptr", bufs=1, space="PSUM"))
                Copy = mybir.ActivationFunctionType.Copy
                oidx = [0]

                def emit_outproj_group(c, ft):
                    ps = ppso.tile([PF, QC], f32, tag="pso", name="pso")
                    for e in range(FG // PF):
                        nc.tensor.matmul(
                            ps[:],
                            wo_sb[e][:, ts(ft, PF)],
                            ctxT_sb[e][:, ts(c, QC)],
                            start=(e == 0),
                            stop=(e == FG // PF - 1),
                        )
                    ot = po_sb.tile([PF, QC], bf16, tag="ot", name="ot")
                    nc.vector.tensor_copy(ot[:], ps[:])
                    oidx[0] += 1
                    eng = nc.sync if oidx[0] % 2 == 0 else nc.gpsimd
                    eng.dma_start(outT_d[ts(ft, PF), ts(c, QC)], ot[:])

                def emit_outproj(c):
                    for ft in range(NF):
                        emit_outproj_group(c, ft)

                while av00[0] < 8:
                    emit_attnv_qs(0, 0, et00, av00[0])
                    av00[0] += 1

                def project_q_late(t, c):
                    ps = ppso.tile([PF, QC], f32, tag="pso", name="pso")
                    for e in range(NE):
                        nc.tensor.matmul(
                            ps[:],
                            wq_sb[e][:, ts(t, PF)],
                            x_sb[e][:, ts(c, QC)],
                            start=(e == 0),
                            stop=(e == NE - 1),
                        )
                    nc.vector.tensor_scalar_add(
                        qt_sb[t][:, ts(c, QC)], ps[:], bq_sb[:, t : t + 1]
                    )

                sc_stream = [(0, 2), (0, 3), (1, 0), (1, 1), (1, 2), (1, 3)]
                av_stream = [(0, 1), (0, 2), (0, 3), (1, 0), (1, 1), (1, 2), (1, 3)]
                ets = {(0, 1): et01}
                pending = [
                    lambda t=t, c=c: project_q_late(t, c)
                    for t in range(2)
                    for c in range(2, NQ)
                ]
                extras = {
                    (0, 3): [
                        lambda: pending.extend(
                            (lambda qt=qt: emit_transpose_qt(qt)) for qt in range(8)
                        )
                    ],
                    (1, 0): [
                        lambda: pending.extend(
                            (lambda ft=ft: emit_outproj_group(0, ft)) for ft in range(NF)
                        )
                    ],
                    (1, 1): [
                        lambda: pending.extend(
                            (lambda ft=ft: emit_outproj_group(1, ft)) for ft in range(NF)
                        )
                    ],
                }
                for i, av in enumerate(av_stream):
                    cur = sc_stream[i] if i < len(sc_stream) else None
                    if cur is not None:
                        ets[cur] = []
                    for kt in range(NK):
                        if cur is not None:
                            emit_scores_kt(cur[0], cur[1], ets[cur], kt)
                        if kt % 2 == 0 and pending:
                            pending.pop(0)()
                        if kt % 2 == 1:
                            emit_attnv_qs(av[0], av[1], ets[av], kt // 2)
                            # tail: peel transposes/out-proj in as soon as
                            # their context tiles complete
                            if av == (1, 3):
                                if kt == 7:
                                    emit_transposes(1, qts=range(8, 12))
                                    pending.extend(
                                        (lambda ft=ft: emit_outproj_group(2, ft))
                                        for ft in range(NF)
                                    )
                                elif kt > 8:
                                    emit_transpose_qt(12 + kt // 2 - 4)
                    for fn in extras.get(av, ()):
                        fn()
                while pending:
                    pending.pop(0)()
                emit_outproj(3)
                _p2.close()

            _stack.close()

    nc.compile()
    _NC_CACHE = nc
    return nc


def _in_maps(x, Wq, bq, Wk, bk, Wv, bv, Wo, bo):
    """Per-core input dicts: core c = b*4 + g."""
    import ml_dtypes

    f = np.float32
    b16 = ml_dtypes.bfloat16
    maps = []
    for b in range(B):
        xT = np.ascontiguousarray(x[b].T).astype(b16)
        for g in range(GROUPS):
            gs = g * FG
            sl = slice(gs, gs + FG)
            maps.append(
                {
                    "xT": xT,
                    "wqT": np.ascontiguousarray(Wq[sl, :].T).astype(b16),
                    "wkT": np.ascontiguousarray(Wk[sl, :].T).astype(b16),
                    "wvT": np.ascontiguousarray(Wv[sl, :].T).astype(b16),
                    "woT": np.ascontiguousarray(Wo[:, sl].T).astype(b16),
                    "bq2": np.ascontiguousarray(bq[sl].reshape(2, PF).T, dtype=f),
                    "bk2": np.ascontiguousarray(bk[sl].reshape(2, PF).T, dtype=f),
                    "bv1": np.ascontiguousarray(bv[sl].reshape(1, FG), dtype=f),
                }
            )
    return maps


def _assemble(results, bo):
    out = np.empty((B, S, E), dtype=np.float32)
    for b in range(B):
        acc = results[b * GROUPS]["outT"].astype(np.float32, copy=True)
        for g in range(1, GROUPS):
            acc += results[b * GROUPS + g]["outT"]
        out[b] = acc.T + bo.astype(np.float32)
    return out


def kernel(x, Wq, bq, Wk, bk, Wv, bv, Wo, bo):
    from concourse.bass_utils import run_bass_kernel_spmd

    nc = _build_nc()
    maps = _in_maps(x, Wq, bq, Wk, bk, Wv, bv, Wo, bo)
    res = run_bass_kernel_spmd(nc, maps, core_ids=list(range(8)))
    return _assemble(res.results, np.asarray(bo))


# revision 29
# speedup vs baseline: 1.0234x; 1.0010x over previous
"""Multi-head attention (B=2, S=2048, E=1024, H=16, D=64) on 8 TRN2 cores.

Sharding: core c = b*4 + g  →  batch b ∈ {0,1}, head-group g ∈ {0..3}
(4 heads = 256 embed columns per group).  Each core computes its group's
Q/K/V projections, attention, and the partial output projection
(out^T [1024, 2048], the Wo[:, group]-contracted context).  Host sums the
4 group partials per batch, transposes, and adds bo.

Layout (matmul inputs bf16, all accumulation fp32 in PSUM):
- x is passed pre-transposed per batch: xT [1024, 2048] (e on partitions).
- Qᵀ/Kᵀ are head-dim-major [256, 2048]: head h of the group lives at
  partition rows (h%2)*64 of tile h//2.
- V is token-major, stored per 128-token tile as [128, 4*65]: head h at
  cols 65h..65h+63, col 65h+64 = 1.0 (ones column).
- scores are computed transposed (k on partitions, q free); softmax has
  no max-subtraction (scores ∈ ±2.8 for this input distribution).
- attn·V: lhsT = expᵀ chunk [k,128q], rhs = [V|1] [k,65] → PSUM [128q, 65]
  whose column 64 is the softmax denominator (per-partition scalar), so
  normalization is reciprocal + tensor_scalar_mul.
- context (q-major) is PE-transposed in 128x128 blocks for the output
  projection.
"""

import sys

import numpy as np

_REPO = "/opt/trn_rl_repo"
if _REPO not in sys.path:
    sys.path.insert(0, _REPO)

B, S, E = 2, 2048, 1024
HEADS, D = 16, 64
GROUPS = 4            # head groups (one per core within a batch)
HG = HEADS // GROUPS  # 4 heads per group
FG = HG * D           # 256 embed columns per group
SCALE = D ** -0.5     # 0.125

PF = 128              # partition tile
QC = 512              # free-dim chunk per matmul
NE = E // PF          # 8 contraction chunks over embed
NQ = S // QC          # 4 q chunks
NK = S // PF          # 16 k tiles
NS = S // PF          # 16 token tiles
NF = E // PF          # 8 output-feature tiles

_NC_CACHE = None


def _build_nc():
    """Build (once) the single-core Bass/Tile program run SPMD on all 8 cores."""
    global _NC_CACHE
    if _NC_CACHE is not None:
        return _NC_CACHE

    import concourse.bass as bass
    import concourse.tile as tile
    from concourse import bacc, mybir
    from concourse.masks import make_identity

    f32 = mybir.dt.float32
    bf16 = mybir.dt.bfloat16
    Exp = mybir.ActivationFunctionType.Exp
    ts = bass.ts

    nc = bacc.Bacc("TRN2", target_bir_lowering=False, debug=False)

    xT_d = nc.declare_dram_parameter("xT", [E, S], bf16, isOutput=False)
    wqT_d = nc.declare_dram_parameter("wqT", [E, FG], bf16, isOutput=False)
    wkT_d = nc.declare_dram_parameter("wkT", [E, FG], bf16, isOutput=False)
    wvT_d = nc.declare_dram_parameter("wvT", [E, FG], bf16, isOutput=False)
    woT_d = nc.declare_dram_parameter("woT", [FG, E], bf16, isOutput=False)
    bq_d = nc.declare_dram_parameter("bq2", [PF, 2], f32, isOutput=False)
    bk_d = nc.declare_dram_parameter("bk2", [PF, 2], f32, isOutput=False)
    bv_d = nc.declare_dram_parameter("bv1", [1, FG], f32, isOutput=False)
    outT_d = nc.declare_dram_parameter("outT", [E, S], bf16, isOutput=True)

    from contextlib import ExitStack

    _stack = ExitStack()
    stack_enter = _stack.enter_context
    with tile.TileContext(nc) as tc:
        with (
            tc.tile_pool(name="w", bufs=1) as pw,
            tc.tile_pool(name="qk", bufs=1) as pqk,
            tc.tile_pool(name="vpool", bufs=1) as pv,
            tc.tile_pool(name="ctx", bufs=1) as pctx,
        ):
            # ---- resident weights / biases -------------------------------
            wq_sb = [pw.tile([PF, FG], bf16, tag=f"wq{e}", name=f"wq{e}") for e in range(NE)]
            wk_sb = [pw.tile([PF, FG], bf16, tag=f"wk{e}", name=f"wk{e}") for e in range(NE)]
            wv_sb = [pw.tile([PF, FG], bf16, tag=f"wv{e}", name=f"wv{e}") for e in range(NE)]
            wo_sb = [pw.tile([PF, E], bf16, tag=f"wo{e}", name=f"wo{e}") for e in range(FG // PF)]
            bq_sb = pw.tile([PF, 2], f32, tag="bq")
            bk_sb = pw.tile([PF, 2], f32, tag="bk")
            bv_row = pw.tile([1, FG], f32, tag="bvr")
            bv_sb = pw.tile([PF, FG], f32, tag="bvf")
            ident = pw.tile([PF, PF], bf16, tag="ident")
            nc.sync.dma_start(bq_sb[:], bq_d[:])
            nc.sync.dma_start(bk_sb[:], bk_d[:])
            nc.sync.dma_start(bv_row[:], bv_d[:])

            # ---- persistent activations ----------------------------------
            qt_sb = [pqk.tile([PF, S], bf16, tag=f"qt{t}", name=f"qt{t}") for t in range(2)]
            # Kᵀ in two zero-padded parity copies so the scores matmul can use
            # K=128 stationary operands (K=64 disables fast weight load):
            # ktz[t] = [even-head rows | 0], kto[t] = [0 | odd-head rows]
            ktz_sb = [pqk.tile([PF, S], bf16, tag=f"ktz{t}", name=f"ktz{t}") for t in range(2)]
            kto_sb = [pqk.tile([PF, S], bf16, tag=f"kto{t}", name=f"kto{t}") for t in range(2)]
            # V token-tiles: [128 tokens, 4 heads x (64 + ones)]
            v_sb = [pv.tile([PF, HG * (D + 1)], bf16, tag=f"v{st}", name=f"v{st}") for st in range(NS)]
            # context, q-major per 128-token tile: [128, 4*64]
            ctx_sb = [pctx.tile([PF, FG], bf16, tag=f"ctx{qt}", name=f"ctx{qt}") for qt in range(NS)]
            # transposed context for the output projection
            ctxT_sb = [pctx.tile([PF, S], bf16, tag=f"ctxT{j}", name=f"ctxT{j}") for j in range(2)]


            # ---- phase 1: Q/K/V projections ------------------------------
            _stack.__enter__()
            pe = stack_enter(tc.tile_pool(name="et", bufs=36))
            pn = stack_enter(tc.tile_pool(name="nrm", bufs=4))
            po_sb = stack_enter(tc.tile_pool(name="owork", bufs=8))
            _p2 = ExitStack()
            _p2.__enter__()
            ppss = _p2.enter_context(tc.tile_pool(name="pss", bufs=2, space="PSUM"))
            _p1 = ExitStack()
            _p1.__enter__()
            px = _p1.enter_context(tc.tile_pool(name="xt", bufs=1))
            pps1 = _p1.enter_context(tc.tile_pool(name="ps1", bufs=2, space="PSUM"))
            pps1v = _p1.enter_context(tc.tile_pool(name="ps1v", bufs=2, space="PSUM"))
            if True:
                x_sb = [px.tile([PF, S], bf16, tag=f"x{e}", name=f"x{e}") for e in range(NE)]
                # interleave K-weight and first x-column DMAs: the first
                # accumulation chain's inputs arrive asap
                # spread the head's DMA issue across idle sequencers —
                # descriptor generation costs ~1.6us per dma_start per queue
                # spread the head's DMA issue across three idle sequencers —
                # descriptor generation costs ~1.6us per dma_start per queue,
                # so the issue rate (not bus bandwidth) paces the head
                for e in range(NE):
                    nc.sync.dma_start(x_sb[e][:, ts(0, QC)], xT_d[ts(e, PF), ts(0, QC)])
                    nc.scalar.dma_start(wk_sb[e][:], wkT_d[ts(e, PF), :])
                    nc.gpsimd.dma_start(wq_sb[e][:], wqT_d[ts(e, PF), :])
                for e in range(NE):
                    eng = nc.sync if e % 2 == 0 else nc.gpsimd
                    eng.dma_start(x_sb[e][:, ts(1, QC)], xT_d[ts(e, PF), ts(1, QC)])
                # setup ops on the pool engine queue after its critical DMAs
                for t in range(2):
                    nc.gpsimd.memset(ktz_sb[t][D:PF, :], 0.0)
                    nc.gpsimd.memset(kto_sb[t][0:D, :], 0.0)
                nc.gpsimd.partition_broadcast(bv_sb[:], bv_row[:])
                make_identity(nc, ident[:])
                for e in range(NE):
                    eng = nc.gpsimd if e % 2 == 0 else nc.sync
                    eng.dma_start(wv_sb[e][:], wvT_d[ts(e, PF), :])
                    eng.dma_start(x_sb[e][:, 2 * QC:4 * QC], xT_d[ts(e, PF), 2 * QC:4 * QC])
                for e in range(FG // PF):
                    nc.sync.dma_start(wo_sb[e][:], woT_d[ts(e, PF), :])

                def project_qk(w_sb, b_sb, o_sb, t, c):
                    ps = pps1.tile([PF, QC], f32, tag="ps1", name="ps1")
                    for e in range(NE):
                        nc.tensor.matmul(
                            ps[:],
                            w_sb[e][:, ts(t, PF)],
                            x_sb[e][:, ts(c, QC)],
                            start=(e == 0),
                            stop=(e == NE - 1),
                        )
                    if o_sb is None:  # K: split into the parity copies
                        nc.vector.tensor_scalar_add(
                            ktz_sb[t][0:D, ts(c, QC)], ps[0:D, :], b_sb[0:D, t : t + 1]
                        )
                        nc.vector.tensor_scalar_add(
                            kto_sb[t][D:PF, ts(c, QC)], ps[D:PF, :], b_sb[D:PF, t : t + 1]
                        )
                    else:
                        nc.vector.tensor_scalar_add(
                            o_sb[t][:, ts(c, QC)], ps[:], b_sb[:, t : t + 1]
                        )

                def emit_scores_kt(half, h, et, kt, split_exp=False):
                    t = h // 2
                    k_sb = ktz_sb[t] if h % 2 == 0 else kto_sb[t]
                    psc = ppss.tile([PF, 2, QC], f32, tag="pss", name="pss")
                    e_t = pe.tile([PF, 2, QC], bf16, tag="et", name="et")
                    for j in range(2):
                        nc.tensor.matmul(
                            psc[:, j, :],
                            k_sb[:, ts(kt, PF)],
                            qt_sb[t][:, ts(half * 2 + j, QC)],
                            start=True,
                            stop=True,
                        )
                        if split_exp:
                            nc.scalar.activation(
                                e_t[:, j, :], psc[:, j, :], Exp, scale=SCALE
                            )
                    if not split_exp:
                        nc.scalar.activation(e_t[:], psc[:], Exp, scale=SCALE)
                    et.append(e_t)

                def emit_scores(half, h, et=None, kts=None):
                    if et is None:
                        et = []
                    for kt in kts if kts is not None else range(NK):
                        emit_scores_kt(half, h, et, kt)
                    return et

                def emit_attnv_qs(half, h, et, qs):
                    j, sub = qs // 4, qs % 4
                    po = ppso.tile([PF, D + 1], f32, tag="pso", name="pso")
                    for kt in range(NK):
                        nc.tensor.matmul(
                            po[:],
                            et[kt][:, j, ts(sub, PF)],
                            v_sb[kt][:, h * (D + 1) : (h + 1) * (D + 1)],
                            start=(kt == 0),
                            stop=(kt == NK - 1),
                        )
                    qt = half * 8 + qs
                    r = pn.tile([PF, 1], f32, tag="r", name="r")
                    nc.vector.reciprocal(r[:], po[:, D : D + 1])
                    nc.vector.tensor_scalar_mul(
                        ctx_sb[qt][:, ts(h, D)], po[:, 0:D], r[:]
                    )

                def emit_attnv(half, h, et):
                    for qs in range(8):
                        emit_attnv_qs(half, h, et, qs)

                def emit_transpose_qt(qt):
                    for j2 in range(2):
                        ptr = pptr.tile([PF, PF], bf16, tag="ptr", name="ptr")
                        nc.tensor.transpose(
                            ptr[:], ctx_sb[qt][:, ts(j2, PF)], ident[:]
                        )
                        nc.vector.tensor_copy(ctxT_sb[j2][:, ts(qt, PF)], ptr[:])

                def emit_transposes(half, qts=None):
                    for qt in qts if qts is not None else range(half * 8, half * 8 + 8):
                        emit_transpose_qt(qt)

                def emit_v(st):
                    ps = pps1v.tile([PF, FG], f32, tag="ps1v", name="ps1v")
                    for e in range(NE):
                        nc.tensor.matmul(
                            ps[:],
                            x_sb[e][:, ts(st, PF)],
                            wv_sb[e][:],
                            start=(e == 0),
                            stop=(e == NE - 1),
                        )
                    nc.vector.memset(v_sb[st][:], 1.0)
                    for h in range(HG):
                        nc.vector.tensor_add(
                            v_sb[st][:, h * (D + 1) : h * (D + 1) + D],
                            ps[:, ts(h, D)],
                            bv_sb[:, ts(h, D)],
                        )

                # interleave K-tile-0 projections with the first combo's
                # scores so the exp stream (ACT, the bottleneck) starts asap;
                # V-projection groups are spread through the chain so all of
                # V is done by the time the first attn-V group runs
                project_qk(wk_sb, bk_sb, None, 0, 0)
                project_qk(wq_sb, bq_sb, qt_sb, 0, 0)
                project_qk(wq_sb, bq_sb, qt_sb, 0, 1)
                et00 = []
                for kt in range(4):
                    emit_scores_kt(0, 0, et00, kt, split_exp=True)
                for c in range(1, NQ):
                    project_qk(wk_sb, bk_sb, None, 0, c)
                    for st in range(4 * (c - 1), 4 * c):
                        emit_v(st)
                    for kt in range(4 * c, 4 * c + 4):
                        emit_scores_kt(0, 0, et00, kt, split_exp=True)
                # interleave combo (0,1) scores (one kt per group) and
                # combo (0,0) attn-V through the phase-1 tail so neither ACT
                # nor PE drains while K-t1/Q-t1/V/Q-rest run
                et01 = []
                av00 = [0]
                tail = (
                    [lambda st=st: emit_v(st) for st in range(12, 16)]
                    + [lambda c=c: project_qk(wk_sb, bk_sb, None, 1, c) for c in range(NQ)]
                    + [lambda c=c: project_qk(wq_sb, bq_sb, qt_sb, 1, c) for c in range(2)]
                )
                for fn in tail:
                    fn()
                    if len(et01) < NK:
                        emit_scores_kt(0, 1, et01, len(et01))
                while len(et01) < NK:
                    emit_scores_kt(0, 1, et01, len(et01))

                # ---- phase 2: attention (pipelined one combo ahead) ------
                _p1.close()
                ppso = _p2.enter_context(tc.tile_pool(name="pso", bufs=3, space="PSUM"))
                pptr = _p2.enter_context(tc.tile_pool(name="ptr", bufs=1, space="PSUM"))
                Copy = mybir.ActivationFunctionType.Copy
                oidx = [0]

                def emit_outproj_group(c, ft):
                    # tail projections use the scores pool's banks (idle by then)
                    if c >= 2:
                        ps = ppss.tile([PF, QC], f32, tag="pss", name="pss")
                    else:
                        ps = ppso.tile([PF, QC], f32, tag="pso", name="pso")
                    for e in range(FG // PF):
                        nc.tensor.matmul(
                            ps[:],
                            wo_sb[e][:, ts(ft, PF)],
                            ctxT_sb[e][:, ts(c, QC)],
                            start=(e == 0),
                            stop=(e == FG // PF - 1),
                        )
                    ot = po_sb.tile([PF, QC], bf16, tag="ot", name="ot")
                    nc.vector.tensor_copy(ot[:], ps[:])
                    oidx[0] += 1
                    eng = nc.sync if oidx[0] % 2 == 0 else nc.gpsimd
                    eng.dma_start(outT_d[ts(ft, PF), ts(c, QC)], ot[:])

                def emit_outproj(c):
                    for ft in range(NF):
                        emit_outproj_group(c, ft)

                while av00[0] < 8:
                    emit_attnv_qs(0, 0, et00, av00[0])
                    av00[0] += 1

                def project_q_late(t, c):
                    ps = ppso.tile([PF, QC], f32, tag="pso", name="pso")
                    for e in range(NE):
                        nc.tensor.matmul(
                            ps[:],
                            wq_sb[e][:, ts(t, PF)],
                            x_sb[e][:, ts(c, QC)],
                            start=(e == 0),
                            stop=(e == NE - 1),
                        )
                    nc.vector.tensor_scalar_add(
                        qt_sb[t][:, ts(c, QC)], ps[:], bq_sb[:, t : t + 1]
                    )

                sc_stream = [(0, 2), (0, 3), (1, 0), (1, 1), (1, 2), (1, 3)]
                av_stream = [(0, 1), (0, 2), (0, 3), (1, 0), (1, 1), (1, 2), (1, 3)]
                ets = {(0, 1): et01}
                pending = [
                    lambda t=t, c=c: project_q_late(t, c)
                    for t in range(2)
                    for c in range(2, NQ)
                ]
                extras = {
                    (0, 3): [
                        lambda: pending.extend(
                            (lambda qt=qt: emit_transpose_qt(qt)) for qt in range(8)
                        )
                    ],
                    (1, 0): [
                        lambda: pending.extend(
                            (lambda ft=ft: emit_outproj_group(0, ft)) for ft in range(NF)
                        )
                    ],
                    (1, 1): [
                        lambda: pending.extend(
                            (lambda ft=ft: emit_outproj_group(1, ft)) for ft in range(NF)
                        )
                    ],
                }
                for i, av in enumerate(av_stream):
                    cur = sc_stream[i] if i < len(sc_stream) else None
                    if cur is not None:
                        ets[cur] = []
                    for kt in range(NK):
                        if cur is not None:
                            emit_scores_kt(cur[0], cur[1], ets[cur], kt)
                        if kt % 2 == 0 and pending:
                            pending.pop(0)()
                        if kt % 2 == 1:
                            emit_attnv_qs(av[0], av[1], ets[av], kt // 2)
                            # tail: peel transposes/out-proj in as soon as
                            # their context tiles complete
                            if av == (1, 3):
                                if kt == 7:
                                    emit_transposes(1, qts=range(8, 12))
                                    pending.extend(
                                        (lambda ft=ft: emit_outproj_group(2, ft))
                                        for ft in range(NF)
                                    )
                                elif kt > 8:
                                    emit_transpose_qt(12 + kt // 2 - 4)
                    for fn in extras.get(av, ()):
                        fn()
                while pending:
                    pending.pop(0)()
                emit_outproj(3)
                _p2.close()

            _stack.close()

    nc.compile()
    _NC_CACHE = nc
    return nc


def _in_maps(x, Wq, bq, Wk, bk, Wv, bv, Wo, bo):
    """Per-core input dicts: core c = b*4 + g."""
    import ml_dtypes

    f = np.float32
    b16 = ml_dtypes.bfloat16
    maps = []
    for b in range(B):
        xT = np.ascontiguousarray(x[b].T).astype(b16)
        for g in range(GROUPS):
            gs = g * FG
            sl = slice(gs, gs + FG)
            maps.append(
                {
                    "xT": xT,
                    "wqT": np.ascontiguousarray(Wq[sl, :].T).astype(b16),
                    "wkT": np.ascontiguousarray(Wk[sl, :].T).astype(b16),
                    "wvT": np.ascontiguousarray(Wv[sl, :].T).astype(b16),
                    "woT": np.ascontiguousarray(Wo[:, sl].T).astype(b16),
                    "bq2": np.ascontiguousarray(bq[sl].reshape(2, PF).T, dtype=f),
                    "bk2": np.ascontiguousarray(bk[sl].reshape(2, PF).T, dtype=f),
                    "bv1": np.ascontiguousarray(bv[sl].reshape(1, FG), dtype=f),
                }
            )
    return maps


def _assemble(results, bo):
    out = np.empty((B, S, E), dtype=np.float32)
    for b in range(B):
        acc = results[b * GROUPS]["outT"].astype(np.float32, copy=True)
        for g in range(1, GROUPS):
            acc += results[b * GROUPS + g]["outT"]
        out[b] = acc.T + bo.astype(np.float32)
    return out


def kernel(x, Wq, bq, Wk, bk, Wv, bv, Wo, bo):
    from concourse.bass_utils import run_bass_kernel_spmd

    nc = _build_nc()
    maps = _in_maps(x, Wq, bq, Wk, bk, Wv, bv, Wo, bo)
    res = run_bass_kernel_spmd(nc, maps, core_ids=list(range(8)))
    return _assemble(res.results, np.asarray(bo))
